# revision 32
# baseline (speedup 1.0000x reference)
"""Causal self-attention Trainium2 kernel (software-pipelined, bf16).

Problem: B=8, T=1024, C=768, H=12 heads, D=64. fp32 in/out.
Sharding: data-parallel over batch - core b computes batch element b.

Per-core dataflow, all operands bf16 (PSUM accumulation stays fp32):

  xT [C, T]                       (host pre-transposed, bf16)
  per head pair g (heads 2g, 2g+1):
    qT[g] [128, T] = wq_g.T @ xT    kT[g] [128, T] = wk_g.T @ xT
  v [T, C] = x @ w_v  -> v_ext [T, H, D+1] with a ones column (row D = denom)
  attention per (g, tq-512-chunk j2, key-chunk-pair c2):
    scoresT[tk, tq] = kT_h x qT_h   (PSUM fp32)
    expT = exp(0.125 * scoresT)     (ScalarE, valid columns only, bf16 out)
    tril-mask diagonal 128x128 sub-blocks (VectorE, strided bf16 TT)
    yT'[65, tq] += v_ext_h.T @ expT (row 64 = softmax denominator)
  normalize: recip(denoms) -> PE broadcast -> one TT per (g, j2)
  out [T, C] = yT.T @ w_proj

The emission order software-pipelines the PE queue: QK projections for
pair g+1 and (later) output-projection chunks are pumped one matmul at
a time between attention steps, so the PE never idles long enough for
the HAM clock gate to re-throttle it, and exp/drain work on ScalarE /
VectorE overlaps PE streaming instead of serializing with it.

Engine budget per head pair (steady state): PE ~7-8us, ScalarE (exp +
qkT drains) ~6-7.5us, VectorE (masks, PSUM drains, recip, normalize)
~5us. PSUM: scores 2x[128,1024] (4 banks) + ps_y 2x[65,512] (2 banks)
+ small 2x[128,512] (2 banks, shared by qk/v/out projections and the
reciprocal broadcast).
"""

import numpy as np
import ml_dtypes

import concourse.bass as bass
import concourse.bacc as bacc
import concourse.tile as tile
from concourse import mybir
from concourse.bass_utils import run_bass_kernel_spmd

N_CORES = 8
T = 1024
C = 768
H = 12
D = 64
P = 128
NK = C // P       # 6 contraction chunks
NT = T // P       # 8 t-chunks
NPAIR = H // 2    # 6 head pairs
F32 = mybir.dt.float32
F32R = mybir.dt.float32r
BF16 = mybir.dt.bfloat16
EXP = mybir.ActivationFunctionType.Exp
COPY = mybir.ActivationFunctionType.Copy
IDENT = mybir.ActivationFunctionType.Identity

NPBF16 = ml_dtypes.bfloat16


def build_kernel(qk_bias=False, v_bias=False, o_bias=False):
    nc = bacc.Bacc("TRN2", target_bir_lowering=False, debug=False,
                   num_devices=N_CORES)

    xT_d = nc.dram_tensor("xT", [C, T], BF16, kind="ExternalInput").ap()
    # column-reordered: pair g occupies cols [256g, 256g+256): first 128 q,
    # next 128 k
    wqk_d = nc.dram_tensor("wqk", [C, 2 * C], BF16, kind="ExternalInput").ap()
    wv_d = nc.dram_tensor("wv", [C, C], BF16, kind="ExternalInput").ap()
    wp_d = nc.dram_tensor("wp", [C, C], BF16, kind="ExternalInput").ap()
    tril_d = nc.dram_tensor("tril", [P, P], BF16, kind="ExternalInput").ap()
    ones64_d = nc.dram_tensor("ones64r", [P, D], F32R,
                              kind="ExternalInput").ap()
    if qk_bias:
        # column j = per-partition bias for output block j (q0,k0,q1,k1,...)
        bqk_d = nc.dram_tensor("bqk_cols", [P, 2 * NPAIR], F32,
                               kind="ExternalInput").ap()
    if v_bias:
        bv_d = nc.dram_tensor("bias_v_b", [P, C], BF16,
                              kind="ExternalInput").ap()
    if o_bias:
        bo_d = nc.dram_tensor("bias_o_b", [P, C], F32,
                              kind="ExternalInput").ap()
    out_d = nc.dram_tensor("out", [T, C], F32, kind="ExternalOutput").ap()

    with tile.TileContext(nc) as tc:
        with tc.tile_pool(name="persist", bufs=1) as pp, \
             tc.tile_pool(name="exp_sb", bufs=4) as te_pool, \
             tc.tile_pool(name="den_sb", bufs=2) as den_pool, \
             tc.tile_pool(name="out_sb", bufs=2) as ot_pool:
            aps = tc.alloc_tile_pool(name="ps", bufs=2, space="PSUM")

            xT = [pp.tile([P, T], BF16, tag=f"xT{k}", name=f"xT{k}")
                  for k in range(NK)]
            wqk = [pp.tile([P, 2 * C], BF16, tag=f"wqk{k}", name=f"wqk{k}")
                   for k in range(NK)]
            wv = [pp.tile([P, C], BF16, tag=f"wv{k}", name=f"wv{k}")
                  for k in range(NK)]
            wp = [pp.tile([P, C], BF16, tag=f"wp{k}", name=f"wp{k}")
                  for k in range(NK)]
            qT = [pp.tile([P, T], BF16, tag=f"qT{g}", name=f"qT{g}")
                  for g in range(NPAIR)]
            kT = [pp.tile([P, T], BF16, tag=f"kT{g}", name=f"kT{g}")
                  for g in range(NPAIR)]
            v_ext = [pp.tile([P, H, D + 1], BF16, tag=f"vext{i}",
                             name=f"vext{i}") for i in range(NT)]
            yT = [pp.tile([P, T], BF16, tag=f"yT{g}", name=f"yT{g}")
                  for g in range(NPAIR)]
            tril = pp.tile([P, P], BF16, tag="tril")
            ones64 = pp.tile([P, D], F32R, tag="ones64")

            # ---------------- input DMAs (need-ordered) ----------------
            # each dma_start costs ~0.7us of issue time on its engine
            # queue, so: few large transfers, alternating between the Sync
            # and (otherwise idle) GpSimd queues
            def dma_eng(n):
                return nc.sync if n % 2 == 0 else nc.gpsimd

            nc.sync.dma_start(tril[:], tril_d[:])
            nc.gpsimd.dma_start(ones64[:], ones64_d[:])
            for k in range(NK):
                dma_eng(k).dma_start(xT[k][:], xT_d[P * k:P * (k + 1), :])
                dma_eng(k + 1).dma_start(wqk[k][:, 0:256],
                                         wqk_d[P * k:P * (k + 1), 0:256])
                dma_eng(k).dma_start(wv[k][:], wv_d[P * k:P * (k + 1), :])
            for i in range(NT):
                nc.gpsimd.memset(v_ext[i][:, :, D:D + 1], 1.0)
            for k in range(NK):
                dma_eng(k).dma_start(wqk[k][:, 256:2 * C],
                                     wqk_d[P * k:P * (k + 1), 256:2 * C])
            for k in range(NK):
                dma_eng(k).dma_start(wp[k][:], wp_d[P * k:P * (k + 1), :])
            if qk_bias:
                bqk = pp.tile([P, 2 * NPAIR], F32, tag="bqk")
                nc.sync.dma_start(bqk[:], bqk_d[:])
            if v_bias:
                bv = pp.tile([P, C], BF16, tag="bv")
                nc.sync.dma_start(bv[:], bv_d[:])
            if o_bias:
                bo = pp.tile([P, C], F32, tag="bo")
                nc.sync.dma_start(bo[:], bo_d[:])

            # PE warmup: dummy matmuls as soon as tril + the first xT half
            # land, so the HAM clock gate is released before real work
            for _ in range(12):
                wps = aps.tile([P, 512], F32, tag="small", name="warm")
                nc.tensor.matmul(wps[:], tril[:], xT[0][:, 0:512],
                                 start=True, stop=True)

            # ---------------- unit generators (PE-pumped) ----------------
            def qkproj_unit(g, which):
                """QK projection for pair g; which: 0 = q, 1 = k.
                Yields once per matmul; emits the ScalarE drain inline."""
                col0 = 256 * g + 128 * which
                dst = (qT if which == 0 else kT)[g]
                for t2 in range(2):
                    ps = aps.tile([P, 512], F32, tag="small", name="ps_qk")
                    for k in range(NK):
                        nc.tensor.matmul(
                            ps[:],
                            wqk[k][:, col0:col0 + P],
                            xT[k][:, 512 * t2:512 * (t2 + 1)],
                            start=(k == 0), stop=(k == NK - 1))
                        yield
                    d = dst[:, 512 * t2:512 * (t2 + 1)]
                    if qk_bias:
                        nc.vector.tensor_scalar_add(
                            out=d, in0=ps[:],
                            scalar1=bqk[:, 2 * g + which:2 * g + which + 1])
                    else:
                        nc.scalar.activation(out=d, in_=ps[:], func=COPY)

            def v_unit(i):
                """v projection for t-chunk i -> v_ext[i] (VectorE drain)."""
                for n2, (c0, c1) in enumerate(((0, 512), (512, 768))):
                    w = c1 - c0
                    ps = aps.tile([P, 512], F32, tag="small", name="ps_v")
                    for k in range(NK):
                        nc.tensor.matmul(
                            ps[:, 0:w],
                            xT[k][:, P * i:P * (i + 1)],
                            wv[k][:, c0:c1],
                            start=(k == 0), stop=(k == NK - 1))
                        yield
                    h0, h1 = c0 // D, c1 // D
                    ps3 = ps[:, 0:w].rearrange("p (h d) -> p h d", d=D)
                    if v_bias:
                        nc.vector.tensor_add(
                            out=v_ext[i][:, h0:h1, 0:D], in0=ps3,
                            in1=bv[:, c0:c1].rearrange("p (h d) -> p h d",
                                                       d=D))
                    else:
                        nc.scalar.activation(out=v_ext[i][:, h0:h1, 0:D],
                                             in_=ps3, func=COPY)

            def proj_unit(i):
                """output projection for t-chunk i (VectorE drain + DMA)."""
                o_t = ot_pool.tile([P, C], F32, tag="out", name="o_t")
                for n2, (c0, c1) in enumerate(((0, 512), (512, 768))):
                    w = c1 - c0
                    ps = aps.tile([P, 512], F32, tag="small", name="ps_o")
                    for g in range(NPAIR):
                        nc.tensor.matmul(
                            ps[:, 0:w],
                            yT[g][:, P * i:P * (i + 1)],
                            wp[g][:, c0:c1],
                            start=(g == 0), stop=(g == NPAIR - 1))
                        yield
                    dst = o_t[:, c0:c1]
                    if o_bias:
                        nc.vector.tensor_add(out=dst, in0=ps[:, 0:w],
                                             in1=bo[:, c0:c1])
                    elif i >= 4:
                        # tail chunks: ScalarE is idle by then
                        nc.scalar.activation(out=dst, in_=ps[:, 0:w],
                                             func=COPY)
                    else:
                        nc.vector.tensor_copy(dst, ps[:, 0:w])
                nc.sync.dma_start(out_d[P * i:P * (i + 1), :], o_t[:])

            fillers = []

            def pump(n):
                for _ in range(n):
                    while fillers:
                        try:
                            next(fillers[0])
                            break
                        except StopIteration:
                            fillers.pop(0)
                    else:
                        break

            def flush():
                pump(1 << 30)

            # ---------------- attention ----------------
            def attn(g, j2, den_t, rbase):
                tq0 = 512 * j2
                n_tk = 4 * (j2 + 1)
                ps_y = [aps.tile([D + 1, 512], F32, tag="psy",
                                 name="ps_y") for _ in range(2)]
                for c2 in range(n_tk // 2):
                    offs = [max(0, P * (2 * c2 + s) - tq0) for s in range(2)]
                    t_e = []
                    for hh in range(2):
                        t_s = aps.tile([P, 1024], F32, tag="big",
                                       name="ps_s")
                        for s in range(2):
                            c = 2 * c2 + s
                            off = offs[s]
                            nc.tensor.matmul(
                                t_s[:, 512 * s + off:512 * (s + 1)],
                                kT[g][D * hh:D * (hh + 1), P * c:P * (c + 1)],
                                qT[g][D * hh:D * (hh + 1),
                                      tq0 + off:tq0 + 512],
                                start=True, stop=True)
                        te = te_pool.tile([P, 1024], BF16, tag="exp",
                                          name="t_e")
                        t_e.append(te)
                        o0 = offs[0]
                        nc.scalar.activation(
                            out=te[:, o0:1024], in_=t_s[:, o0:1024],
                            func=EXP, scale=0.125)
                        if offs[1] > 0:
                            # mask the two diagonal 128x128 sub-blocks with
                            # one strided TT against tril
                            sl = bass.AP(
                                tensor=te.tensor,
                                offset=te.offset + o0,
                                ap=[te.ap[0], [512 + P, 2], [1, P]])
                            trb = bass.AP(
                                tensor=tril.tensor,
                                offset=tril.offset,
                                ap=[tril.ap[0], [0, 2], [1, P]])
                            nc.vector.tensor_mul(out=sl, in0=sl, in1=trb)
                        pump(3)
                    for hh in range(2):
                        h = 2 * g + hh
                        for s in range(2):
                            c = 2 * c2 + s
                            off = offs[s]
                            nc.tensor.matmul(
                                ps_y[hh][:, off:512],
                                v_ext[c][:, h, :],
                                t_e[hh][:, 512 * s + off:512 * (s + 1)],
                                start=(c2 == 0 and s == 0),
                                stop=(c == n_tk - 1))
                    pump(3)
                # drain yT (unnormalized) and stash this pair's two
                # denominator rows at 32-aligned partitions of den_t
                for hh in range(2):
                    dst = yT[g][D * hh:D * (hh + 1), tq0:tq0 + 512]
                    nc.vector.tensor_copy(dst, ps_y[hh][0:D, :])
                    r = rbase + 32 * hh
                    nc.vector.tensor_copy(den_t[r:r + 1, :],
                                          ps_y[hh][D:D + 1, :])

            def norm_pairs(j2, glist, den_t):
                """Batched softmax normalization for the pairs in glist:
                one reciprocal over their denominator rows, then per head
                a PE broadcast + one TT multiply into yT."""
                tq0 = 512 * j2
                np_ = 64 * len(glist)
                recd = den_pool.tile([P, 512], F32, tag="recd", name="recd")
                nc.vector.reciprocal_approx_fast(out=recd[0:np_, :],
                                                 in_=den_t[0:np_, :])
                recr = den_pool.tile([P, 512], F32R, tag="recr", name="recr")
                nc.vector.tensor_copy(recr[0:np_, :], recd[0:np_, :])
                for gg, g in enumerate(glist):
                    for hh in range(2):
                        r = 32 * (2 * gg + hh)
                        bc = aps.tile([P, 512], F32, tag="small", name="bc")
                        nc.tensor.matmul(
                            bc[0:D, :],
                            ones64[r:r + 1, :],
                            recr[r:r + 1, :],
                            start=True, stop=True,
                            tile_position=(r, 0) if r == 96 else None)
                        dst = yT[g][D * hh:D * (hh + 1), tq0:tq0 + 512]
                        nc.vector.tensor_mul(out=dst, in0=bc[0:D, :],
                                             in1=dst)

            # ---------------- schedule ----------------
            for it in qkproj_unit(0, 0):
                pass
            for it in qkproj_unit(0, 1):
                pass
            for i in range(NT):
                for it in v_unit(i):
                    pass
            fillers.append(qkproj_unit(1, 0))
            fillers.append(qkproj_unit(1, 1))
            den_t = None
            for g in range(NPAIR):
                if g % 2 == 0:
                    den_t = den_pool.tile([P, 512], F32, tag="den",
                                          name="den")
                attn(g, 0, den_t, rbase=64 * (g % 2))
                if g % 2 == 1:
                    norm_pairs(0, [g - 1, g], den_t)
                # flush so qT/kT (g+1) are fully emitted before
                # attn(g+1, 0) begins, then queue pair g+2's projections
                flush()
                if g + 2 < NPAIR:
                    fillers.append(qkproj_unit(g + 2, 0))
                    fillers.append(qkproj_unit(g + 2, 1))
            for i in range(4):
                fillers.append(proj_unit(i))
            for g in range(NPAIR):
                # per-pair normalization in the second round so the last
                # pair's norm chain does not serialize the output tail
                den_t = den_pool.tile([P, 512], F32, tag="den", name="den")
                attn(g, 1, den_t, rbase=0)
                norm_pairs(1, [g], den_t)
            flush()
            for i in range(4, NT):
                for it in proj_unit(i):
                    pass
            aps.release()

    nc.compile()
    return nc


_NC_CACHE = {}


def _get_nc(qk_bias, v_bias, o_bias):
    key = (qk_bias, v_bias, o_bias)
    if key not in _NC_CACHE:
        _NC_CACHE[key] = build_kernel(*key)
    return _NC_CACHE[key]


def make_in_maps(x, w_attn, b_attn, w_proj, b_proj, qk_bias, v_bias, o_bias):
    # reorder qk weight columns: pair g -> [q cols | k cols]
    wqk_re = np.empty((C, 2 * C), dtype=np.float32)
    for g in range(NPAIR):
        wqk_re[:, 256 * g:256 * g + 128] = w_attn[:, 128 * g:128 * (g + 1)]
        wqk_re[:, 256 * g + 128:256 * (g + 1)] = \
            w_attn[:, C + 128 * g:C + 128 * (g + 1)]
    # tril[tk, l] = 1 iff l >= tk (keep query-pos >= key-pos)
    tril = np.triu(np.ones((P, P), dtype=np.float32))

    shared = {
        "wqk": wqk_re.astype(NPBF16),
        "wv": np.ascontiguousarray(w_attn[:, 2 * C:]).astype(NPBF16),
        "wp": np.ascontiguousarray(w_proj).astype(NPBF16),
        "tril": tril.astype(NPBF16),
        "ones64r": np.ones((P, D), dtype=np.float32),
    }
    if qk_bias:
        bq = np.empty((P, 2 * NPAIR), dtype=np.float32)
        for g in range(NPAIR):
            bq[:, 2 * g] = b_attn[128 * g:128 * (g + 1)]
            bq[:, 2 * g + 1] = b_attn[C + 128 * g:C + 128 * (g + 1)]
        shared["bqk_cols"] = bq
    if v_bias:
        shared["bias_v_b"] = np.broadcast_to(
            b_attn[2 * C:], (P, C)).astype(NPBF16)
    if o_bias:
        shared["bias_o_b"] = np.ascontiguousarray(
            np.broadcast_to(b_proj, (P, C)).astype(np.float32))
    in_maps = []
    for b in range(N_CORES):
        m = dict(shared)
        m["xT"] = np.ascontiguousarray(x[b].T).astype(NPBF16)
        in_maps.append(m)
    return in_maps


def run(x, w_attn, b_attn, w_proj, b_proj, **spmd_kwargs):
    x = np.asarray(x, dtype=np.float32)
    w_attn = np.asarray(w_attn, dtype=np.float32)
    b_attn = np.asarray(b_attn, dtype=np.float32)
    w_proj = np.asarray(w_proj, dtype=np.float32)
    b_proj = np.asarray(b_proj, dtype=np.float32)
    qk_bias = bool(np.any(b_attn[:2 * C]))
    v_bias = bool(np.any(b_attn[2 * C:]))
    o_bias = bool(np.any(b_proj))
    nc = _get_nc(qk_bias, v_bias, o_bias)
    in_maps = make_in_maps(x, w_attn, b_attn, w_proj, b_proj,
                           qk_bias, v_bias, o_bias)
    res = run_bass_kernel_spmd(nc, in_maps, core_ids=list(range(N_CORES)),
                               **spmd_kwargs)
    out = np.stack([res.results[b]["out"] for b in range(N_CORES)], axis=0)
    return out.astype(np.float32), res


def kernel(x, w_attn, b_attn, w_proj, b_proj):
    out, _ = run(x, w_attn, b_attn, w_proj, b_proj)
    return out
